# revision 20
# baseline (speedup 1.0000x reference)
"""Block-diagonal rotation (COB) kernel for Trainium2, 8 NeuronCores.

Computes out[..., block_i] = x[..., block_i] @ W_i.T for 8 square blocks of
sizes [512, 1024, 256, 768, 384, 640, 128, 384] (features sum to 4096),
x shape (4, 2048, 4096) fp32.

Strategy (bf16 end-to-end, data-parallel over rows, W-stationary):
  - 8192 rows split 8 ways (1024 rows/core); each core holds all weights.
  - The HOST pre-transposes x per core and packs it in PE-consumption
    order; it also unscrambles the outT blocks the device returns.  The
    device never transposes: the PE computes outT[n, m] = sum_d W[n, d]
    * xT[d, m] with 128x128 W chunks stationary and xT streaming 512
    rows per matmul.  328 matmuls/core, all N=512: 167,936 PE cycles =
    70 us at 2.4 GHz -- the bf16 streaming floor for this op.
  - Loop order is k-OUTER (d-chunk stages) with all of a block's
    n-chunk PSUM groups accumulating concurrently, so a block's first
    matmul only needs its first k-tiles -- input demand is smooth at
    the 128-KiB-tile level, no per-block prefetch cliffs (which
    previously caused mid-kernel HAM re-throttles).
  - Per block: m1 row-half first (consumes x only, ~75-150 GB/s), then
    m0 (consumes w + x, ~220-450 GB/s); block order starts with tiny b6
    (first matmul after ~0.3 MiB of DMA), then big blocks first so the
    per-block average input demand (~150 GB/s) stays far under supply.
  - Input is host-packed into 34 large contiguous DMAs (x: 16 pairs of
    k-tiles, [128, 2048] 512 KiB; w: 18 per-block pair-slices), issued
    on the sync ring in consumption order.  Each dma_start costs
    ~600 ns of issue time on its engine, so the previous 107-DMA
    version was issue-limited to ~210 GB/s; 34 keeps the ring ahead.
  - PSUM results are downcast-copied to bf16 staging (alternating
    ACT/DVE); one output DRAM tensor per (m-half, block) holds the
    staging tile verbatim (stores up to 1 MiB on the scalar ring); the
    host reassembles.  bf16 end-to-end keeps HBM traffic at 21.1
    MiB/core; rel err ~3.9e-3 vs the 2e-2 gate.
"""

import numpy as np
import ml_dtypes

import concourse.bacc as bacc
import concourse.mybir as mybir
from concourse.tile import TileContext
from concourse.bass_utils import run_bass_kernel_spmd

SIZES = [512, 1024, 256, 768, 384, 640, 128, 384]
OFFS = np.cumsum([0] + SIZES)
N_CORES = 8
ROWS_TOTAL = 4 * 2048
ROWS_PER_CORE = ROWS_TOTAL // N_CORES  # 1024
D = 4096
P = 128
M_SLICE = 512                      # rows per PSUM pass (one fp32 bank)
N_MSL = ROWS_PER_CORE // M_SLICE   # 2
KT = D // P                        # 32 global 128-feature chunks

BF16 = mybir.dt.bfloat16
F32 = mybir.dt.float32

# block processing order: tiny b6 first (instant PE start), then big
# blocks first (lowest input-demand rate early), small ones at the end
BO = [6, 1, 3, 5, 0, 4, 7, 2]

# k-tile consumption order and pairing for the packed x feed
K_ORDER = []
for _b in BO:
    K_ORDER.extend(range(int(OFFS[_b]) // P, int(OFFS[_b + 1]) // P))
K_POS = {k: i for i, k in enumerate(K_ORDER)}
N_XQUAD = KT // 4  # 8 quad-tiles of 4 k-tiles each

_cache = {}


def build_nc():
    if "nc" in _cache:
        return _cache["nc"]
    nc = bacc.Bacc()
    # x feed: 8 quad-tiles [128, 4096]; quad i = k-tiles K_ORDER[4i..4i+3];
    # within a tile, cols a*1024 + m*512 .. +512 hold k-tile a's m-half rows
    xt_d = nc.declare_dram_parameter("xt", [N_XQUAD * P, 8 * M_SLICE], BF16,
                                     isOutput=False)
    # w feed per block: [128, nk*s]; cols k*s + j*128 .. hold the
    # stationary chunk for (d-chunk k, n-chunk j)
    w_d = [
        nc.declare_dram_parameter(f"w{i}", [P, (s // P) * s], BF16, isOutput=False)
        for i, s in enumerate(SIZES)
    ]
    o_d = {
        (m, b): nc.declare_dram_parameter(
            f"o{m}_{b}", [P, (SIZES[b] // P) * M_SLICE], BF16, isOutput=True
        )
        for m in range(N_MSL)
        for b in range(len(SIZES))
    }

    xt_v = xt_d.rearrange("(i p) c -> i p c", p=P)

    with TileContext(nc) as tc:
        with (
            tc.tile_pool(name="wres", bufs=1) as wres,
            tc.tile_pool(name="xres", bufs=1) as xres,
            tc.tile_pool(name="osb", bufs=1) as osb,
            tc.tile_pool(name="mm", bufs=8, space="PSUM") as mm_p,
        ):
            # --- PE warm-up: dummy matmuls on a memset scratch tile so the
            # HAM clock gate opens during the DMA prologue, before real work
            scr = osb.tile([P, M_SLICE], BF16, tag="warm")
            nc.vector.memset(scr[:], 0)
            wps = mm_p.tile([P, M_SLICE], F32, tag="mm", name="warmps")
            for _ in range(14):
                nc.tensor.matmul(wps[:], scr[:, :P], scr[:], start=True,
                                 stop=True)

            # --- input DMAs: all on the sync ring, w pair-slices
            # interleaved with the x quads per k-stage in consumption
            # order (every matmul needs BOTH the w chunk and the x tile).
            # Stores ride the gpsimd SWDGE ring, which has its own
            # flow-control semaphores, so store completions (gated on
            # compute) never block input issue.
            xtiles = {}
            wtile = {}

            # x is delivered as 16 pair-units (512 KiB) filling halves of
            # the 8 quad tiles, interleaved with w pair-slices at matching
            # k-stage granularity
            def emit_xunit(u):
                i = u // 2
                if i not in xtiles:
                    xtiles[i] = xres.tile([P, 8 * M_SLICE], BF16,
                                          tag=f"xq{i}", name="xqt")
                h = (u % 2) * 4 * M_SLICE
                nc.sync.dma_start(out=xtiles[i][:, h:h + 4 * M_SLICE],
                                  in_=xt_v[i][:, h:h + 4 * M_SLICE])

            xptr = {"i": 0}
            for b in BO:
                s = SIZES[b]
                nk = s // P
                g0 = int(OFFS[b]) // P
                wt = wres.tile([P, nk * s], BF16, tag=f"w{b}", name="wtt")
                wtile[b] = wt
                for q in range(0, nk, 2):
                    hi = min(q + 2, nk)
                    last_pos = max(K_POS[g0 + k] for k in range(q, hi))
                    while xptr["i"] * 2 <= last_pos:
                        emit_xunit(xptr["i"])
                        xptr["i"] += 1
                    nc.sync.dma_start(out=wt[:, q * s:hi * s],
                                      in_=w_d[b][:, q * s:hi * s])
            while xptr["i"] < 2 * N_XQUAD:
                emit_xunit(xptr["i"])
                xptr["i"] += 1

            def xsl(m, k):
                pos = K_POS[k]
                return xtiles[pos // 4][
                    :, (pos % 4) * 2 * M_SLICE + m * M_SLICE:
                       (pos % 4) * 2 * M_SLICE + (m + 1) * M_SLICE]

            # --- compute: per block, m1 pass then m0 pass, k-outer ---
            cp = {"i": 0, "s": 0}

            def process(b, m):
                s = SIZES[b]
                nk = s // P
                g0 = int(OFFS[b]) // P
                ps = {}
                for k in range(nk):
                    for j in range(nk):
                        if k == 0:
                            ps[j] = mm_p.tile([P, M_SLICE], F32, tag="mm", name="mmps")
                        nc.tensor.matmul(
                            ps[j][:],
                            wtile[b][:, k * s + j * P:k * s + (j + 1) * P],
                            xsl(m, g0 + k),
                            start=(k == 0),
                            stop=(k == nk - 1),
                        )
                last = b == 6 and m == 0
                stage = osb.tile([P, nk * M_SLICE], BF16, tag=f"os{b}")
                for j in range(nk):
                    dst = stage[:, j * M_SLICE:(j + 1) * M_SLICE]
                    if last or cp["i"] % 2 != 0:
                        # final copy on DVE: the scalar engine is draining
                        # the previous block's copies/stores right then
                        nc.vector.tensor_copy(dst, ps[j][:])
                    else:
                        nc.scalar.copy(dst, ps[j][:])
                    cp["i"] += 1
                # bulk stores on the SWDGE ring (own flow control, no
                # latency requirement); the last few on the HWDGE rings,
                # which are idle by then and issue in ~0.6 us -- SWDGE
                # issue is ~1-2 us per DMA and was serializing the drain.
                # The final two stores take the sync ring so they don't
                # queue behind the scalar ones.
                if cp["s"] >= 14:
                    nc.sync.dma_start(out=o_d[(m, b)][:, :], in_=stage[:])
                elif cp["s"] >= 11:
                    nc.scalar.dma_start(out=o_d[(m, b)][:, :], in_=stage[:])
                else:
                    nc.gpsimd.dma_start(out=o_d[(m, b)][:, :], in_=stage[:])
                cp["s"] += 1

            for b in BO:
                process(b, 1)
                if b != 6:
                    process(b, 0)
            # b6's m0 pass (one matmul, one copy, 128 KiB store) runs
            # last so the kernel drains on the smallest possible tail
            process(6, 0)

    nc.finalize()
    _cache["nc"] = nc
    return nc


def build_in_maps(x, w0, w1, w2, w3, w4, w5, w6, w7):
    x = np.asarray(x, dtype=np.float32).reshape(ROWS_TOTAL, D)
    xb = x.astype(ml_dtypes.bfloat16)
    ws = [w0, w1, w2, w3, w4, w5, w6, w7]
    # w feed: [128, nk*s] with cols k*s.. = W.T rows k*128..(k+1)*128
    wfs = []
    for w in ws:
        s = w.shape[0]
        nk = s // P
        wt = np.ascontiguousarray(np.asarray(w, dtype=np.float32).T).astype(
            ml_dtypes.bfloat16
        )
        wfs.append(
            np.ascontiguousarray(
                wt.reshape(nk, P, s).transpose(1, 0, 2).reshape(P, nk * s)
            )
        )
    korder = np.array(K_ORDER)
    in_maps = []
    for c in range(N_CORES):
        xc = xb[c * ROWS_PER_CORE:(c + 1) * ROWS_PER_CORE]  # [1024, 4096]
        xT = np.ascontiguousarray(xc.T)                      # [4096, 1024]
        tiles = xT.reshape(KT, P, ROWS_PER_CORE)             # [32, 128, 1024]
        # pair i: [2, 128, 1024] -> [128, 2, 1024] -> [128, 2048]
        xf = (
            tiles[korder]
            .reshape(N_XQUAD, 4, P, ROWS_PER_CORE)
            .transpose(0, 2, 1, 3)
            .reshape(N_XQUAD * P, 8 * M_SLICE)
        )
        m_ = {"xt": np.ascontiguousarray(xf)}
        for i, wf in enumerate(wfs):
            m_[f"w{i}"] = wf
        in_maps.append(m_)
    return in_maps


def kernel(x, w0, w1, w2, w3, w4, w5, w6, w7):
    nc = build_nc()
    in_maps = build_in_maps(x, w0, w1, w2, w3, w4, w5, w6, w7)
    res = run_bass_kernel_spmd(nc, in_maps, list(range(N_CORES)))
    out = np.empty([ROWS_TOTAL, D], dtype=np.float32)
    for c in range(N_CORES):
        rows = out[c * ROWS_PER_CORE:(c + 1) * ROWS_PER_CORE]
        for m in range(N_MSL):
            for b, s in enumerate(SIZES):
                nk = s // P
                o = res.results[c][f"o{m}_{b}"]  # [128, nk*512] bf16
                # o[p, j*512 + r] = outT[OFFS[b] + j*128 + p, m*512 + r]
                blk = (
                    o.reshape(P, nk, M_SLICE)
                    .transpose(1, 0, 2)
                    .reshape(s, M_SLICE)
                )
                rows[m * M_SLICE:(m + 1) * M_SLICE, OFFS[b]:OFFS[b] + s] = blk.T
    return out.reshape(4, 2048, D)


# revision 24
# speedup vs baseline: 1.0016x; 1.0016x over previous
"""Block-diagonal rotation (COB) kernel for Trainium2, 8 NeuronCores.

Computes out[..., block_i] = x[..., block_i] @ W_i.T for 8 square blocks of
sizes [512, 1024, 256, 768, 384, 640, 128, 384] (features sum to 4096),
x shape (4, 2048, 4096) fp32.

Strategy (bf16 end-to-end, data-parallel over rows, W-stationary):
  - 8192 rows split 8 ways (1024 rows/core); each core holds all weights.
  - The HOST pre-transposes x per core and packs it in PE-consumption
    order; it also unscrambles the outT blocks the device returns.  The
    device never transposes: the PE computes outT[n, m] = sum_d W[n, d]
    * xT[d, m] with 128x128 W chunks stationary and xT streaming 512
    rows per matmul.  328 matmuls/core, all N=512: 167,936 PE cycles =
    70 us at 2.4 GHz -- the bf16 streaming floor for this op.
  - Loop order is k-OUTER (d-chunk stages) with all of a block's
    n-chunk PSUM groups accumulating concurrently, so a block's first
    matmul only needs its first k-tiles -- input demand is smooth at
    the 128-KiB-tile level, no per-block prefetch cliffs (which
    previously caused mid-kernel HAM re-throttles).
  - Per block: m1 row-half first (consumes x only, ~75-150 GB/s), then
    m0 (consumes w + x, ~220-450 GB/s); block order starts with tiny b6
    (first matmul after ~0.3 MiB of DMA), then big blocks first so the
    per-block average input demand (~150 GB/s) stays far under supply.
  - Input is host-packed into 34 large contiguous DMAs (x: 16 pairs of
    k-tiles, [128, 2048] 512 KiB; w: 18 per-block pair-slices), issued
    on the sync ring in consumption order.  Each dma_start costs
    ~600 ns of issue time on its engine, so the previous 107-DMA
    version was issue-limited to ~210 GB/s; 34 keeps the ring ahead.
  - PSUM results are downcast-copied to bf16 staging (alternating
    ACT/DVE); one output DRAM tensor per (m-half, block) holds the
    staging tile verbatim (stores up to 1 MiB on the scalar ring); the
    host reassembles.  bf16 end-to-end keeps HBM traffic at 21.1
    MiB/core; rel err ~3.9e-3 vs the 2e-2 gate.
"""

import numpy as np
import ml_dtypes

import concourse.bacc as bacc
import concourse.mybir as mybir
from concourse.tile import TileContext
from concourse.bass_utils import run_bass_kernel_spmd

SIZES = [512, 1024, 256, 768, 384, 640, 128, 384]
OFFS = np.cumsum([0] + SIZES)
N_CORES = 8
ROWS_TOTAL = 4 * 2048
ROWS_PER_CORE = ROWS_TOTAL // N_CORES  # 1024
D = 4096
P = 128
M_SLICE = 512                      # rows per PSUM pass (one fp32 bank)
N_MSL = ROWS_PER_CORE // M_SLICE   # 2
KT = D // P                        # 32 global 128-feature chunks

BF16 = mybir.dt.bfloat16
F32 = mybir.dt.float32

# block processing order: tiny b6 first (instant PE start), then big
# blocks first (lowest input-demand rate early), small ones at the end
BO = [6, 1, 3, 5, 0, 4, 7, 2]

# k-tile consumption order and pairing for the packed x feed
K_ORDER = []
for _b in BO:
    K_ORDER.extend(range(int(OFFS[_b]) // P, int(OFFS[_b + 1]) // P))
K_POS = {k: i for i, k in enumerate(K_ORDER)}
N_XQUAD = KT // 4  # 8 quad-tiles of 4 k-tiles each

_cache = {}


def build_nc():
    if "nc" in _cache:
        return _cache["nc"]
    nc = bacc.Bacc()
    # x feed: 8 quad-tiles [128, 4096]; quad i = k-tiles K_ORDER[4i..4i+3];
    # within a tile, cols a*1024 + m*512 .. +512 hold k-tile a's m-half rows
    xt_d = nc.declare_dram_parameter("xt", [N_XQUAD * P, 8 * M_SLICE], BF16,
                                     isOutput=False)
    # w feed per block: [128, nk*s]; cols k*s + j*128 .. hold the
    # stationary chunk for (d-chunk k, n-chunk j)
    w_d = [
        nc.declare_dram_parameter(f"w{i}", [P, (s // P) * s], BF16, isOutput=False)
        for i, s in enumerate(SIZES)
    ]
    o_d = {
        (m, b): nc.declare_dram_parameter(
            f"o{m}_{b}", [P, (SIZES[b] // P) * M_SLICE], BF16, isOutput=True
        )
        for m in range(N_MSL)
        for b in range(len(SIZES))
    }

    xt_v = xt_d.rearrange("(i p) c -> i p c", p=P)

    with TileContext(nc) as tc:
        with (
            tc.tile_pool(name="wres", bufs=1) as wres,
            tc.tile_pool(name="xres", bufs=1) as xres,
            tc.tile_pool(name="osb", bufs=1) as osb,
            tc.tile_pool(name="mm", bufs=8, space="PSUM") as mm_p,
        ):
            # --- PE warm-up: dummy matmuls on a memset scratch tile so the
            # HAM clock gate opens during the DMA prologue, before real work
            scr = osb.tile([P, M_SLICE], BF16, tag="warm")
            nc.vector.memset(scr[:], 0)
            wps = mm_p.tile([P, M_SLICE], F32, tag="mm", name="warmps")
            for _ in range(8):
                nc.tensor.matmul(wps[:], scr[:, :P], scr[:], start=True,
                                 stop=True)

            # --- input DMAs: all on the sync ring, w pair-slices
            # interleaved with the x quads per k-stage in consumption
            # order (every matmul needs BOTH the w chunk and the x tile).
            # Stores ride the gpsimd SWDGE ring, which has its own
            # flow-control semaphores, so store completions (gated on
            # compute) never block input issue.
            xtiles = {}
            wtile = {}

            # x quad tile layout: [p, m-half, slot, 512].  The m1 halves
            # stream first as 256-KiB pair-units interleaved with w
            # pair-slices (they gate each block's m1 pass); the m0 halves
            # follow as 512-KiB quad-units with a full pass of slack, so
            # the early-window input demand drops from ~300 to ~225 GB/s.
            def xtile(i):
                if i not in xtiles:
                    xtiles[i] = xres.tile([P, 8 * M_SLICE], BF16,
                                          tag=f"xq{i}", name="xqt")
                return xtiles[i]

            def emit_m1unit(u):
                t = xtile(u // 2)
                h = 4 * M_SLICE + (u % 2) * 2 * M_SLICE
                nc.sync.dma_start(out=t[:, h:h + 2 * M_SLICE],
                                  in_=xt_v[u // 2][:, h:h + 2 * M_SLICE])

            def emit_m0quad(i):
                t = xtile(i)
                nc.sync.dma_start(out=t[:, :4 * M_SLICE],
                                  in_=xt_v[i][:, :4 * M_SLICE])

            m1p = {"i": 0}
            m0p = {"i": 0}
            for b in BO:
                s = SIZES[b]
                nk = s // P
                g0 = int(OFFS[b]) // P
                wt = wres.tile([P, nk * s], BF16, tag=f"w{b}", name="wtt")
                wtile[b] = wt
                for q in range(0, nk, 2):
                    hi = min(q + 2, nk)
                    last_pos = max(K_POS[g0 + k] for k in range(q, hi))
                    while m1p["i"] * 2 <= last_pos:
                        emit_m1unit(m1p["i"])
                        m1p["i"] += 1
                    nc.sync.dma_start(out=wt[:, q * s:hi * s],
                                      in_=w_d[b][:, q * s:hi * s])
                # m0 quads whose m1 units are fully emitted; consumed no
                # earlier than this block's m0 pass
                while m0p["i"] * 4 + 3 < m1p["i"] * 2:
                    emit_m0quad(m0p["i"])
                    m0p["i"] += 1
            while m0p["i"] < N_XQUAD:
                emit_m0quad(m0p["i"])
                m0p["i"] += 1

            def xsl(m, k):
                pos = K_POS[k]
                c0 = m * 4 * M_SLICE + (pos % 4) * M_SLICE
                return xtiles[pos // 4][:, c0:c0 + M_SLICE]

            # --- compute: per block, m1 pass then m0 pass, k-outer ---
            cp = {"i": 0, "s": 0}

            def process(b, m):
                s = SIZES[b]
                nk = s // P
                g0 = int(OFFS[b]) // P
                ps = {}
                for k in range(nk):
                    for j in range(nk):
                        if k == 0:
                            ps[j] = mm_p.tile([P, M_SLICE], F32, tag="mm", name="mmps")
                        nc.tensor.matmul(
                            ps[j][:],
                            wtile[b][:, k * s + j * P:k * s + (j + 1) * P],
                            xsl(m, g0 + k),
                            start=(k == 0),
                            stop=(k == nk - 1),
                        )
                last = b == 6 and m == 0
                stage = osb.tile([P, nk * M_SLICE], BF16, tag=f"os{b}")
                for j in range(nk):
                    dst = stage[:, j * M_SLICE:(j + 1) * M_SLICE]
                    if last or cp["i"] % 2 != 0:
                        # final copy on DVE: the scalar engine is draining
                        # the previous block's copies/stores right then
                        nc.vector.tensor_copy(dst, ps[j][:])
                    else:
                        nc.scalar.copy(dst, ps[j][:])
                    cp["i"] += 1
                # bulk stores on the SWDGE ring (own flow control, no
                # latency requirement); the last few on the HWDGE rings,
                # which are idle by then and issue in ~0.6 us -- SWDGE
                # issue is ~1-2 us per DMA and was serializing the drain.
                # The final two stores take the sync ring so they don't
                # queue behind the scalar ones.
                if cp["s"] >= 14:
                    nc.sync.dma_start(out=o_d[(m, b)][:, :], in_=stage[:])
                elif cp["s"] >= 11:
                    nc.scalar.dma_start(out=o_d[(m, b)][:, :], in_=stage[:])
                else:
                    nc.gpsimd.dma_start(out=o_d[(m, b)][:, :], in_=stage[:])
                cp["s"] += 1

            for b in BO:
                process(b, 1)
                if b != 6:
                    process(b, 0)
            # b6's m0 pass (one matmul, one copy, 128 KiB store) runs
            # last so the kernel drains on the smallest possible tail
            process(6, 0)

    nc.finalize()
    _cache["nc"] = nc
    return nc


def build_in_maps(x, w0, w1, w2, w3, w4, w5, w6, w7):
    x = np.asarray(x, dtype=np.float32).reshape(ROWS_TOTAL, D)
    xb = x.astype(ml_dtypes.bfloat16)
    ws = [w0, w1, w2, w3, w4, w5, w6, w7]
    # w feed: [128, nk*s] with cols k*s.. = W.T rows k*128..(k+1)*128
    wfs = []
    for w in ws:
        s = w.shape[0]
        nk = s // P
        wt = np.ascontiguousarray(np.asarray(w, dtype=np.float32).T).astype(
            ml_dtypes.bfloat16
        )
        wfs.append(
            np.ascontiguousarray(
                wt.reshape(nk, P, s).transpose(1, 0, 2).reshape(P, nk * s)
            )
        )
    korder = np.array(K_ORDER)
    in_maps = []
    for c in range(N_CORES):
        xc = xb[c * ROWS_PER_CORE:(c + 1) * ROWS_PER_CORE]  # [1024, 4096]
        xT = np.ascontiguousarray(xc.T)                      # [4096, 1024]
        tiles = xT.reshape(KT, P, ROWS_PER_CORE)             # [32, 128, 1024]
        # pair i: [2, 128, 1024] -> [128, 2, 1024] -> [128, 2048]
        # quad tile layout [quad, p, m-half, slot, 512]
        xf = (
            tiles[korder]
            .reshape(N_XQUAD, 4, P, 2, M_SLICE)
            .transpose(0, 2, 3, 1, 4)
            .reshape(N_XQUAD * P, 8 * M_SLICE)
        )
        m_ = {"xt": np.ascontiguousarray(xf)}
        for i, wf in enumerate(wfs):
            m_[f"w{i}"] = wf
        in_maps.append(m_)
    return in_maps


def kernel(x, w0, w1, w2, w3, w4, w5, w6, w7):
    nc = build_nc()
    in_maps = build_in_maps(x, w0, w1, w2, w3, w4, w5, w6, w7)
    res = run_bass_kernel_spmd(nc, in_maps, list(range(N_CORES)))
    out = np.empty([ROWS_TOTAL, D], dtype=np.float32)
    for c in range(N_CORES):
        rows = out[c * ROWS_PER_CORE:(c + 1) * ROWS_PER_CORE]
        for m in range(N_MSL):
            for b, s in enumerate(SIZES):
                nk = s // P
                o = res.results[c][f"o{m}_{b}"]  # [128, nk*512] bf16
                # o[p, j*512 + r] = outT[OFFS[b] + j*128 + p, m*512 + r]
                blk = (
                    o.reshape(P, nk, M_SLICE)
                    .transpose(1, 0, 2)
                    .reshape(s, M_SLICE)
                )
                rows[m * M_SLICE:(m + 1) * M_SLICE, OFFS[b]:OFFS[b] + s] = blk.T
    return out.reshape(4, 2048, D)
